# revision 23
# baseline (speedup 1.0000x reference)
"""GAT-style edge-softmax aggregation kernel for Trainium2 (8 NeuronCores).

Reference computation (D=128, N_SRC=N_DST=50000, E=640000):
    hs = feat_src @ W_src + b_src
    hd = feat_dst @ W_dst + b_dst
    e  = hs[src_idx] + hd[dst_idx]
    s  = lrelu(lrelu(e) @ W_a1 + b_a1) @ W_a2 + b_a2
    alpha = segment_softmax(s, dst_idx)
    n_f = segment_sum(alpha * e, dst_idx)
    out = lrelu(n_f @ W_out + b_out)

Sharding: edges partitioned by destination node; core c owns dst rows
[c*6250, (c+1)*6250).  Edges are sorted by dst on the host; each core's
edge stream is organized into 49 windows of 128 consecutive dst nodes,
each window padded to a uniform tile count T_W so all 8 cores execute a
single SPMD instruction stream.  Scores skip the segment-max subtraction
(|s| <= ~1 for this distribution, exp is exact in fp32).
"""

import sys

sys.path.insert(0, "/opt/trn_rl_repo")

import numpy as np

import concourse.bacc as bacc
import concourse.bass as bass
import concourse.mybir as mybir
import concourse.tile as tile
from concourse import bass_utils
from concourse.masks import make_identity

P = 128
D = 128
N_CORES = 8
N_SRC = 50000
N_DST = 50000
E = 640000
DST_PER_CORE = N_DST // N_CORES          # 6250
WINDOWS_PER_CORE = (DST_PER_CORE + P - 1) // P   # 49 (48 full + 106)
NEG_SLOPE = 0.01
GROUP_TILES = 4                           # tiles (of 128 edges) per MLP group
SG_WINDOWS = 2                            # windows gathered per supergroup
F32 = mybir.dt.float32
F32R = mybir.dt.float32r
I32 = mybir.dt.int32


# ----------------------------------------------------------------------------
# Host-side schedule
# ----------------------------------------------------------------------------

SPLIT = 25000  # hs table split (int16 gather indices must stay < 32768)


def _wrap16(flat_i16: np.ndarray) -> np.ndarray:
    """dma_gather index layout: element i -> partition i%16, col i//16,
    zero-padded to 128 partitions (sim bounds-checks all 128)."""
    n = len(flat_i16)
    assert n % 16 == 0
    blk = flat_i16.reshape(n // 16, 16).T   # [16, n/16]
    return np.ascontiguousarray(np.tile(blk, (8, 1)))  # replicated to 128p


def build_schedule(src_idx: np.ndarray, dst_idx: np.ndarray):
    """Sort edges by dst; core c owns dst [c*6250, (c+1)*6250) split into 49
    windows of 128 dst.  Within each window edges are split into A (src <
    SPLIT) and B (src >= SPLIT) parts so hs gathers use int16 indices into
    half-tables.  Every (core, window) is padded to the same A_TILES/B_TILES
    so all cores share one SPMD instruction stream.

    Window slot layout: [A edges | pad ... (A_TILES*128) | B edges | pad ...]
    Pad slots gather row 0 (finite data) and have dstloc=300 (one-hot zero).
    Returns (a_tiles, b_tiles, per_core) with per-core arrays:
      idx_lo [128, 49*A_TILES*8] i16, idx_hi [128, 49*B_TILES*8] i16,
      idx_hd [128, 49*T_W*8] i16, dstloc [128, n_tiles] f32.
    """
    order = np.argsort(dst_idx, kind="stable")
    s_sorted = src_idx[order].astype(np.int64)
    d_sorted = dst_idx[order].astype(np.int64)
    starts = np.searchsorted(d_sorted, np.arange(0, N_DST + 1))

    wins = []   # (core, window) -> (srcA list, srcB list, dlocA, dlocB, dhd..)
    max_a = max_b = 0
    for c in range(N_CORES):
        base_dst = c * DST_PER_CORE
        cw = []
        for w in range(WINDOWS_PER_CORE):
            lo = base_dst + w * P
            hi = min(lo + P, base_dst + DST_PER_CORE)
            a, b = starts[lo], starts[hi]
            s = s_sorted[a:b]
            d = d_sorted[a:b]
            mA = s < SPLIT
            cw.append((s[mA], d[mA] - base_dst, d[mA] - lo,
                       s[~mA] - SPLIT, d[~mA] - base_dst, d[~mA] - lo))
            max_a = max(max_a, int(mA.sum()))
            max_b = max(max_b, int((~mA).sum()))
        wins.append(cw)
    a_tiles = (max_a + P - 1) // P
    b_tiles = (max_b + P - 1) // P
    t_w = a_tiles + b_tiles
    n_tiles = WINDOWS_PER_CORE * t_w

    per_core = []
    for c in range(N_CORES):
        ilo = np.zeros((WINDOWS_PER_CORE, a_tiles * P), np.int16)
        ihi = np.zeros((WINDOWS_PER_CORE, b_tiles * P), np.int16)
        ihd = np.zeros((WINDOWS_PER_CORE, t_w * P), np.int16)
        dloc = np.full((WINDOWS_PER_CORE, t_w * P), 300.0, np.float32)
        for w in range(WINDOWS_PER_CORE):
            sA, hdA, dlA, sB, hdB, dlB = wins[c][w]
            nA, nB = len(sA), len(sB)
            ilo[w, :nA] = sA
            ihi[w, :nB] = sB
            ihd[w, :nA] = hdA
            ihd[w, a_tiles * P:a_tiles * P + nB] = hdB
            dloc[w, :nA] = dlA
            dloc[w, a_tiles * P:a_tiles * P + nB] = dlB
        per_core.append({
            "idx_lo": _wrap16(ilo.ravel()),
            "idx_hi": _wrap16(ihi.ravel()),
            "idx_hd": _wrap16(ihd.ravel()),
            "dstloc": np.ascontiguousarray(
                dloc.reshape(n_tiles, P).T.astype(np.float32)),
        })
    return a_tiles, b_tiles, per_core


# ----------------------------------------------------------------------------
# Device kernel
# ----------------------------------------------------------------------------

def _unwrap16(arr: np.ndarray) -> np.ndarray:
    """Inverse of _wrap16: [128, n/16] -> flat [n]."""
    return np.ascontiguousarray(arr[0:16, :].T.reshape(-1)).astype(np.int64)


def reconstruct_streams(a_tiles: int, b_tiles: int, pc: dict):
    """Rebuild flat per-slot (src_row, hd_row, dstloc) streams for testing."""
    t_w = a_tiles + b_tiles
    lo = _unwrap16(pc["idx_lo"]).reshape(WINDOWS_PER_CORE, a_tiles * P)
    hi = _unwrap16(pc["idx_hi"]).reshape(WINDOWS_PER_CORE, b_tiles * P)
    hd = _unwrap16(pc["idx_hd"])
    gs = np.concatenate([np.concatenate([lo[w], hi[w] + SPLIT])
                         for w in range(WINDOWS_PER_CORE)])
    dl = pc["dstloc"].T.ravel()
    return gs, hd, dl


def _proj_table(nc, tc, sbuf, psum, table_dram, feat_dram, w_sb, bias_col,
                n_rows, ident):
    """table = feat @ W + b, row layout, via PE transpose->matmul->transpose."""
    n0 = 0
    while n0 < n_rows:
        nn = min(512, n_rows - n0)
        f_t = sbuf.tile([P, 512], F32, tag="pa_f")
        n_sub = (nn + P - 1) // P
        for j in range(n_sub):
            w = min(P, nn - j * P)
            nc.sync.dma_start(
                out=f_t[0:w, j * P:j * P + P],
                in_=feat_dram[n0 + j * P:n0 + j * P + w, :])
        ps_t = psum.tile([P, 512], F32, tag="pa_ps1")
        for j in range(n_sub):
            w = min(P, nn - j * P)
            nc.tensor.transpose(
                out=ps_t[:, j * P:j * P + w],
                in_=f_t[0:w, j * P:j * P + P],
                identity=ident[0:w, 0:w])
        ft_sb = sbuf.tile([P, 512], F32, tag="pa_ft")
        nc.scalar.activation(ft_sb[:, :nn], ps_t[:, :nn],
                             mybir.ActivationFunctionType.Identity)
        ps_h = psum.tile([P, 512], F32, tag="pa_ps2")
        nc.tensor.matmul(
            out=ps_h[:, :nn],
            lhsT=w_sb[:, :],
            rhs=ft_sb[:, :nn],
            start=True, stop=True)
        ht_sb = sbuf.tile([P, 512], F32, tag="pa_ht")
        nc.scalar.activation(ht_sb[:, :nn], ps_h[:, :nn],
                             mybir.ActivationFunctionType.Identity,
                             bias=bias_col[:, :])
        ps_r = psum.tile([P, 512], F32, tag="pa_ps3")
        for j in range(n_sub):
            w = min(P, nn - j * P)
            nc.tensor.transpose(
                out=ps_r[0:w, j * P:j * P + P],
                in_=ht_sb[:, j * P:j * P + w],
                identity=ident[:, :])
        hr_sb = sbuf.tile([P, 512], F32, tag="pa_hr")
        for j in range(n_sub):
            w = min(P, nn - j * P)
            nc.vector.tensor_copy(hr_sb[0:w, j * P:j * P + P],
                                  ps_r[0:w, j * P:j * P + P])
        for j in range(n_sub):
            w = min(P, nn - j * P)
            nc.sync.dma_start(
                out=table_dram[n0 + j * P:n0 + j * P + w, :],
                in_=hr_sb[0:w, j * P:j * P + P])
        n0 += nn


def build_kernel(a_tiles: int, b_tiles: int, stage: str = "full"):
    t_w = a_tiles + b_tiles
    n_tiles = WINDOWS_PER_CORE * t_w
    nc = bacc.Bacc("TRN2", target_bir_lowering=False, debug=False,
                   enable_asserts=False, num_devices=N_CORES)

    feat_src = nc.dram_tensor("feat_src", [N_SRC, D], F32, kind="ExternalInput")
    fd_loc = nc.dram_tensor("fd_loc", [DST_PER_CORE, D], F32, kind="ExternalInput")
    idx_lo = nc.dram_tensor("idx_lo", [P, WINDOWS_PER_CORE * a_tiles * 8],
                            mybir.dt.int16, kind="ExternalInput")
    idx_hi = nc.dram_tensor("idx_hi", [P, WINDOWS_PER_CORE * b_tiles * 8],
                            mybir.dt.int16, kind="ExternalInput")
    idx_hd = nc.dram_tensor("idx_hd", [P, WINDOWS_PER_CORE * t_w * 8],
                            mybir.dt.int16, kind="ExternalInput")
    dstloc = nc.dram_tensor("dstloc", [P, n_tiles], F32, kind="ExternalInput")
    w_src = nc.dram_tensor("w_src", [D, D], F32, kind="ExternalInput")
    w_dst = nc.dram_tensor("w_dst", [D, D], F32, kind="ExternalInput")
    w_a1 = nc.dram_tensor("w_a1", [D, D], F32, kind="ExternalInput")
    w_a2 = nc.dram_tensor("w_a2", [D, 1], F32, kind="ExternalInput")
    w_out = nc.dram_tensor("w_out", [D, D], F32, kind="ExternalInput")
    bsrc_c = nc.dram_tensor("bsrc_c", [D, 1], F32, kind="ExternalInput")
    bdst_c = nc.dram_tensor("bdst_c", [D, 1], F32, kind="ExternalInput")
    ba1_c = nc.dram_tensor("ba1_c", [D, 1], F32, kind="ExternalInput")
    ba2_c = nc.dram_tensor("ba2_c", [D, 1], F32, kind="ExternalInput")
    bout_c = nc.dram_tensor("bout_c", [D, 1], F32, kind="ExternalInput")
    iota_row = nc.dram_tensor("iota_row", [P, P], F32, kind="ExternalInput")
    out_loc = nc.dram_tensor("out_loc", [DST_PER_CORE, D], F32,
                             kind="ExternalOutput")

    LRELU = mybir.ActivationFunctionType.Lrelu
    IDENT = mybir.ActivationFunctionType.Identity
    EXP = mybir.ActivationFunctionType.Exp

    with tile.TileContext(nc) as tc:
        with (
            tc.tile_pool(name="const", bufs=1) as cpool,
            tc.tile_pool(name="dram", bufs=1, space="DRAM") as dpool,
        ):
            ident = cpool.tile([P, P], F32)
            make_identity(nc, ident[:])
            iota_sb = cpool.tile([P, P], F32)
            nc.sync.dma_start(out=iota_sb[:], in_=iota_row[:])

            def load_const(dr, shape, tag):
                t = cpool.tile(list(shape), F32, tag=tag)
                nc.sync.dma_start(out=t[:], in_=dr[:])
                return t

            wsrc_sb = load_const(w_src, (D, D), "c_wsrc")
            wdst_sb = load_const(w_dst, (D, D), "c_wdst")
            wa1_sb = load_const(w_a1, (D, D), "c_wa1")
            wa2_sb = load_const(w_a2, (D, 1), "c_wa2")
            wout_sb = load_const(w_out, (D, D), "c_wout")
            bsrc_sb = load_const(bsrc_c, (D, 1), "c_bsrc")
            bdst_sb = load_const(bdst_c, (D, 1), "c_bdst")
            ba1_sb = load_const(ba1_c, (D, 1), "c_ba1")
            ba2_sb = load_const(ba2_c, (D, 1), "c_ba2")
            bout_sb = load_const(bout_c, (D, 1), "c_bout")

            hs_dram = dpool.tile([N_SRC, D], F32)
            hd_dram = dpool.tile([DST_PER_CORE, D], F32)

            # ---------------- Phase A: node projections -> DRAM tables ----
            with (
                tc.tile_pool(name="pa_sbuf", bufs=2) as pa_sbuf,
                tc.tile_pool(name="pa_psum", bufs=2, space="PSUM") as pa_psum,
            ):
                _proj_table(nc, tc, pa_sbuf, pa_psum, hs_dram[:], feat_src,
                            wsrc_sb, bsrc_sb, N_SRC, ident)
                _proj_table(nc, tc, pa_sbuf, pa_psum, hd_dram[:], fd_loc,
                            wdst_sb, bdst_sb, DST_PER_CORE, ident)

            if stage == "A":
                with tc.tile_pool(name="dbg", bufs=2) as dbg:
                    for j in range(0, DST_PER_CORE, P):
                        w = min(P, DST_PER_CORE - j)
                        t = dbg.tile([P, P], F32, tag="dbg_t")
                        nc.sync.dma_start(out=t[0:w, :], in_=hs_dram[j:j + w, :])
                        nc.sync.dma_start(out=out_loc[j:j + w, :], in_=t[0:w, :])

            # winbuf: per-window [num(128) | denom] accumulators
            winbuf = cpool.tile([P, WINDOWS_PER_CORE * 129], F32)

            # ---------------- Phase B: edge pipeline ----------------------
            with (
                tc.tile_pool(name="gat_sbuf", bufs=2) as gb,
                tc.tile_pool(name="grp_sbuf", bufs=3) as gsb,
                tc.tile_pool(name="oh_sbuf", bufs=4) as ohp,
                tc.tile_pool(name="ps_xe", bufs=2, space="PSUM") as ps_xe,
                tc.tile_pool(name="ps_h1", bufs=1, space="PSUM") as ps_h1,
                tc.tile_pool(name="ps_sc", bufs=1, space="PSUM") as ps_sc,
                tc.tile_pool(name="ps_win", bufs=2, space="PSUM") as ps_win,
            ):
                sg_tiles = SG_WINDOWS * t_w
                win_ps = None
                t0 = 0 if stage != "A" else n_tiles
                while t0 < n_tiles:
                    k = min(sg_tiles, n_tiles - t0)
                    w0 = t0 // t_w
                    nw = k // t_w
                    ilo_sb = gb.tile([P, SG_WINDOWS * a_tiles * 8],
                                     mybir.dt.int16, tag="ilo")
                    ihi_sb = gb.tile([P, SG_WINDOWS * b_tiles * 8],
                                     mybir.dt.int16, tag="ihi")
                    ihd_sb = gb.tile([P, SG_WINDOWS * t_w * 8],
                                     mybir.dt.int16, tag="ihd")
                    dl_sb = gb.tile([P, sg_tiles], F32, tag="dl")
                    nc.sync.dma_start(
                        out=ilo_sb[:, :nw * a_tiles * 8],
                        in_=idx_lo[:, w0 * a_tiles * 8:(w0 + nw) * a_tiles * 8])
                    nc.sync.dma_start(
                        out=ihi_sb[:, :nw * b_tiles * 8],
                        in_=idx_hi[:, w0 * b_tiles * 8:(w0 + nw) * b_tiles * 8])
                    nc.sync.dma_start(
                        out=ihd_sb[:, :nw * t_w * 8],
                        in_=idx_hd[:, w0 * t_w * 8:(w0 + nw) * t_w * 8])
                    nc.sync.dma_start(out=dl_sb[:, :k], in_=dstloc[:, t0:t0 + k])
                    xs = gb.tile([P, sg_tiles * P], F32, tag="xs")
                    xd = gb.tile([P, sg_tiles * P], F32, tag="xd")
                    xs_v = xs[:].rearrange("p (t f) -> p t f", f=P)
                    xd_v = xd[:].rearrange("p (t f) -> p t f", f=P)
                    for i in range(nw):
                        nc.gpsimd.dma_gather(
                            out_ap=xs_v[:, i * t_w:i * t_w + a_tiles, :],
                            in_ap=hs_dram[0:SPLIT, :],
                            idxs_ap=ilo_sb[:, i * a_tiles * 8:(i + 1) * a_tiles * 8],
                            num_idxs=a_tiles * P,
                            num_idxs_reg=a_tiles * P,
                            elem_size=P, single_packet=False)
                        nc.gpsimd.dma_gather(
                            out_ap=xs_v[:, i * t_w + a_tiles:(i + 1) * t_w, :],
                            in_ap=hs_dram[SPLIT:N_SRC, :],
                            idxs_ap=ihi_sb[:, i * b_tiles * 8:(i + 1) * b_tiles * 8],
                            num_idxs=b_tiles * P,
                            num_idxs_reg=b_tiles * P,
                            elem_size=P, single_packet=False)
                        nc.gpsimd.dma_gather(
                            out_ap=xd_v[:, i * t_w:(i + 1) * t_w, :],
                            in_ap=hd_dram[:],
                            idxs_ap=ihd_sb[:, i * t_w * 8:(i + 1) * t_w * 8],
                            num_idxs=t_w * P,
                            num_idxs_reg=t_w * P,
                            elem_size=P, single_packet=False)

                    if stage == "G":
                        nc.sync.dma_start(
                            out=out_loc[:].rearrange("a b -> (a b)")[
                                0:sg_tiles * P * P],
                            in_=xs[:])
                        break

                    g0 = 0
                    while g0 < k:
                        gt = min(GROUP_TILES, k - g0)   # tiles in this group
                        ne = gt * P                     # edges in group
                        xe = gsb.tile([P, GROUP_TILES * 129], F32, tag="xe")
                        xe_v = xe[:].rearrange("p (t c) -> p t c", c=129)
                        nc.vector.memset(xe_v[:, 0:gt, 128:129], 1.0)
                        xs_v = xs[:, g0 * P:(g0 + gt) * P].rearrange(
                            "p (t f) -> p t f", f=P)
                        xd_v = xd[:, g0 * P:(g0 + gt) * P].rearrange(
                            "p (t f) -> p t f", f=P)
                        nc.vector.tensor_tensor(
                            out=xe_v[:, 0:gt, 0:128], in0=xs_v, in1=xd_v,
                            op=mybir.AluOpType.add)

                        xet = ps_xe.tile([P, 512], F32, tag="xet")
                        for t in range(gt):
                            nc.tensor.transpose(
                                out=xet[:, t * P:(t + 1) * P],
                                in_=xe_v[:, t, 0:128],
                                identity=ident[:])
                        lt = gsb.tile([P, 512], F32, tag="lt")
                        nc.scalar.activation(lt[:, :ne], xet[:, :ne], LRELU,
                                             alpha=NEG_SLOPE)
                        h1 = ps_h1.tile([P, 512], F32, tag="h1")
                        nc.tensor.matmul(
                            out=h1[:, :ne],
                            lhsT=wa1_sb[:],
                            rhs=lt[:, :ne],
                            start=True, stop=True)
                        a1 = gsb.tile([P, 512], F32, tag="a1")
                        nc.scalar.activation(a1[:, :ne], h1[:, :ne], LRELU,
                                             bias=ba1_sb[:], alpha=NEG_SLOPE)
                        sc = ps_sc.tile([P, GROUP_TILES], F32, tag="sc")
                        for t in range(gt):
                            nc.tensor.matmul(
                                out=sc[:, t:t + 1],
                                lhsT=a1[:, t * P:(t + 1) * P],
                                rhs=wa2_sb[:],
                                start=True, stop=True,
                                skip_group_check=True)
                        exc = gsb.tile([P, GROUP_TILES], F32, tag="exc")
                        nc.scalar.activation(exc[:, :gt], sc[:, :gt], EXP,
                                             bias=ba2_sb[:])
                        for t in range(gt):
                            tg = t0 + g0 + t           # global tile index
                            w = tg // t_w              # window index
                            j = tg % t_w
                            if j == 0:
                                win_ps = ps_win.tile([P, 129], F32, tag="win")
                            oh = ohp.tile([P, P], F32, tag="oh")
                            nc.vector.tensor_scalar(
                                out=oh[:],
                                in0=iota_sb[:],
                                scalar1=dl_sb[:, g0 + t:g0 + t + 1],
                                scalar2=exc[:, t:t + 1],
                                op0=mybir.AluOpType.is_equal,
                                op1=mybir.AluOpType.mult)
                            nc.tensor.matmul(
                                out=win_ps[:],
                                lhsT=oh[:],
                                rhs=xe_v[:, t, 0:129],
                                start=(j == 0), stop=(j == t_w - 1))
                            if j == t_w - 1:
                                nc.scalar.activation(
                                    winbuf[:, w * 129:(w + 1) * 129],
                                    win_ps[:], IDENT)
                        g0 += gt
                    t0 += k

            if stage == "B":
                nc.sync.dma_start(
                    out=out_loc[:].rearrange("a b -> (a b)"),
                    in_=winbuf[:, 0:6250])

            # ---------------- Phase C: normalize + output projection ------
            with (
                tc.tile_pool(name="pc_sbuf", bufs=2) as pc,
                tc.tile_pool(name="pc_psum", bufs=2, space="PSUM") as pcp,
            ):
                w0 = 0 if stage == "full" else WINDOWS_PER_CORE
                while w0 < WINDOWS_PER_CORE:
                    nw = min(4, WINDOWS_PER_CORE - w0)
                    nfn = pc.tile([P, 4 * P], F32, tag="nfn")
                    for i in range(nw):
                        w = w0 + i
                        dn = pc.tile([P, 1], F32, tag="dn")
                        # eps keeps empty-dst denominators inside the DVE
                        # reciprocal's valid range (real denoms are >= 0.37)
                        nc.vector.tensor_scalar_add(
                            dn[:], winbuf[:, w * 129 + 128:w * 129 + 129],
                            1e-6)
                        rc = pc.tile([P, 1], F32, tag="rc")
                        nc.vector.reciprocal(rc[:], dn[:])
                        nc.vector.tensor_scalar_mul(
                            nfn[:, i * P:(i + 1) * P],
                            winbuf[:, w * 129:w * 129 + 128],
                            rc[:])
                    ps_t = pcp.tile([P, 512], F32, tag="pc_ps1")
                    for i in range(nw):
                        nc.tensor.transpose(
                            out=ps_t[:, i * P:(i + 1) * P],
                            in_=nfn[:, i * P:(i + 1) * P],
                            identity=ident[:])
                    nft = pc.tile([P, 4 * P], F32, tag="nft")
                    nc.vector.tensor_copy(nft[:, :nw * P], ps_t[:, :nw * P])
                    ps_o = pcp.tile([P, 512], F32, tag="pc_ps2")
                    nc.tensor.matmul(
                        out=ps_o[:, :nw * P],
                        lhsT=wout_sb[:],
                        rhs=nft[:, :nw * P],
                        start=True, stop=True)
                    ot = pc.tile([P, 4 * P], F32, tag="ot")
                    nc.scalar.activation(ot[:, :nw * P], ps_o[:, :nw * P],
                                         LRELU, bias=bout_sb[:],
                                         alpha=NEG_SLOPE)
                    ps_r = pcp.tile([P, 512], F32, tag="pc_ps3")
                    for i in range(nw):
                        nc.tensor.transpose(
                            out=ps_r[:, i * P:(i + 1) * P],
                            in_=ot[:, i * P:(i + 1) * P],
                            identity=ident[:])
                    orow = pc.tile([P, 4 * P], F32, tag="orow")
                    nc.vector.tensor_copy(orow[:, :nw * P], ps_r[:, :nw * P])
                    for i in range(nw):
                        w = w0 + i
                        lo = w * P
                        hi = min(lo + P, DST_PER_CORE)
                        nc.sync.dma_start(
                            out=out_loc[lo:hi, :],
                            in_=orow[0:hi - lo, i * P:(i + 1) * P])
                    w0 += nw

    nc.compile()
    return nc


_KERNEL_CACHE = {}
TRACE = False
TRACE_CORES = [0]
LAST_RESULT = None


def kernel(feat_src, feat_dst, src_idx, dst_idx,
           W_src, b_src, W_dst, b_dst,
           W_a1, b_a1, W_a2, b_a2, W_out, b_out):
    feat_src = np.ascontiguousarray(np.asarray(feat_src, dtype=np.float32))
    feat_dst = np.ascontiguousarray(np.asarray(feat_dst, dtype=np.float32))
    src_idx = np.asarray(src_idx, dtype=np.int32)
    dst_idx = np.asarray(dst_idx, dtype=np.int32)

    a_tiles, b_tiles, per_core = build_schedule(src_idx, dst_idx)

    key = (a_tiles, b_tiles)
    if key not in _KERNEL_CACHE:
        _KERNEL_CACHE[key] = build_kernel(a_tiles, b_tiles)
    nc = _KERNEL_CACHE[key]

    def col(v):
        return np.ascontiguousarray(
            np.asarray(v, dtype=np.float32).reshape(D, 1))

    ba2col = np.full((D, 1), np.float32(np.asarray(b_a2).reshape(-1)[0]),
                     dtype=np.float32)
    iota = np.ascontiguousarray(
        np.tile(np.arange(P, dtype=np.float32), (P, 1)))

    common = {
        "feat_src": feat_src,
        "w_src": np.ascontiguousarray(np.asarray(W_src, np.float32)),
        "w_dst": np.ascontiguousarray(np.asarray(W_dst, np.float32)),
        "w_a1": np.ascontiguousarray(np.asarray(W_a1, np.float32)),
        "w_a2": np.ascontiguousarray(
            np.asarray(W_a2, np.float32).reshape(D, 1)),
        "w_out": np.ascontiguousarray(np.asarray(W_out, np.float32)),
        "bsrc_c": col(b_src), "bdst_c": col(b_dst), "ba1_c": col(b_a1),
        "ba2_c": ba2col, "bout_c": col(b_out), "iota_row": iota,
    }
    in_maps = []
    for c in range(N_CORES):
        m = dict(common)
        m["fd_loc"] = np.ascontiguousarray(
            feat_dst[c * DST_PER_CORE:(c + 1) * DST_PER_CORE, :])
        m.update(per_core[c])
        in_maps.append(m)

    res = bass_utils.run_bass_kernel_spmd(nc, in_maps,
                                          core_ids=list(range(N_CORES)),
                                          trace=TRACE,
                                          trace_cores=TRACE_CORES if TRACE else None)
    global LAST_RESULT
    LAST_RESULT = res
    out = np.concatenate([res.results[c]["out_loc"] for c in range(N_CORES)],
                         axis=0)
    return out


if __name__ == "__main__":
    rng = np.random.default_rng(0)
    fs = rng.standard_normal((N_SRC, D), dtype=np.float32)
    print("self-test stub; use test.py")
